# revision 8
# baseline (speedup 1.0000x reference)
"""Trainium2 Bass kernel for nn_ContrastLoss_Disentangle.

Contract: kernel(**inputs) takes the FULL (unsharded) inputs and returns the
same structure the reference returns: (loss_label, loss_norm, loss_triple)
as float32 scalars.

Pipeline (8 NeuronCores, data-parallel):
  host:    pose norms + normalization + [C*D, Np] transpose (poseFT)
  device1: per-core: nlp-row sumsq (ACT square+accum), raw nlp.pose dots
           (DVE tensor_tensor_reduce), and a [512, 1024] block of the
           pm gram matrix (PE, f32r fast path or bf16 hi/lo 3-pass)
  host:    scores + BCE, pm assembly + stable argsort rank-select (furthest)
  device2: per-core dots of the gathered "hard positive" nlp rows
  host:    triplet loss assembly
"""

import os
import numpy as np

import concourse.bass as bass
import concourse.tile as tile
from concourse import bacc, mybir
from concourse.bass2jax import install_neuronx_cc_hook, partition_id_tensor, _bass_exec_p

C, NP, K, D = 8, 2048, 4, 256
NN = NP * K          # 8192
NCORES = 8
NPL = NP // NCORES   # 256 poses per core
NNL = NN // NCORES   # 1024 nlp rows per core
NT = NNL // 128      # 8 nlp tiles per category per core
CD = C * D           # 2048 contraction size

# pm block grid: 4 row-blocks x 2 col-blocks
PM_MI, PM_NJ = 4, 2
PM_M = NP // PM_MI   # 512 rows per core block
PM_N = NP // PM_NJ   # 1024 cols per core block

PM_MODE = os.environ.get("PM_MODE", "f32r")  # "f32r" | "bf16split" | "f32"

_runners = {}


def _build_dots_kernel(with_pm: bool):
    """Per-core program. Inputs (per core):
      nlp   [C, NNL, D] f32   (raw nlp rows of this core)
      pose  [C, NPL, D] f32   (normalized pose rows matching this core's nlps)
      pm_l / pm_r             (poseFT column slices; only when with_pm)
    Outputs:
      ssq  [128, C*NT] f32    sumsq of nlp rows  (col = cat*NT + t)
      rdot [128, C*NT] f32    dot(nlp_row, poseF[row//4])
      pmblk [PM_M, PM_N] f32  (only when with_pm)
    """
    nc = bacc.Bacc("TRN2", target_bir_lowering=False, debug=False,
                   num_devices=NCORES)
    nlp = nc.dram_tensor("nlp", [C, NNL, D], mybir.dt.float32,
                         kind="ExternalInput").ap()
    pose = nc.dram_tensor("pose", [C, NPL, D], mybir.dt.float32,
                          kind="ExternalInput").ap()
    if with_pm:
        if PM_MODE == "bf16split":
            pml_h = nc.dram_tensor("pml_h", [CD, PM_M], mybir.dt.bfloat16, kind="ExternalInput").ap()
            pml_l = nc.dram_tensor("pml_l", [CD, PM_M], mybir.dt.bfloat16, kind="ExternalInput").ap()
            pmr_h = nc.dram_tensor("pmr_h", [CD, PM_N], mybir.dt.bfloat16, kind="ExternalInput").ap()
            pmr_l = nc.dram_tensor("pmr_l", [CD, PM_N], mybir.dt.bfloat16, kind="ExternalInput").ap()
        else:
            pmdt = mybir.dt.float32r if PM_MODE == "f32r" else mybir.dt.float32
            pm_l = nc.dram_tensor("pm_l", [CD, PM_M], pmdt, kind="ExternalInput").ap()
            pm_r = nc.dram_tensor("pm_r", [CD, PM_N], pmdt, kind="ExternalInput").ap()
        pmblk = nc.dram_tensor("pmblk", [PM_M, PM_N], mybir.dt.float32,
                               kind="ExternalOutput").ap()
    ssq = nc.dram_tensor("ssq", [128, C * NT], mybir.dt.float32,
                         kind="ExternalOutput").ap()
    rdot = nc.dram_tensor("rdot", [128, C * NT], mybir.dt.float32,
                          kind="ExternalOutput").ap()

    with tile.TileContext(nc) as tc:
        with tc.tile_pool(name="io", bufs=3) as io, \
             tc.tile_pool(name="pose_p", bufs=2) as pose_p, \
             tc.tile_pool(name="bcast", bufs=3) as bcast, \
             tc.tile_pool(name="scr", bufs=3) as scr, \
             tc.tile_pool(name="accum", bufs=1) as accum, \
             tc.tile_pool(name="matres", bufs=1) as matres, \
             tc.tile_pool(name="rhs_p", bufs=3) as rhs_p, \
             tc.tile_pool(name="ev", bufs=3) as ev, \
             tc.tile_pool(name="ps", bufs=1, space="PSUM") as ps:

            ssq_t = accum.tile([128, C * NT], mybir.dt.float32, tag="ssq")
            rdot_t = accum.tile([128, C * NT], mybir.dt.float32, tag="rdot")
            nc.gpsimd.memset(ssq_t[:], 0.0)
            nc.gpsimd.memset(rdot_t[:], 0.0)

            # ---- A) sumsq + dots over the nlp shard -----------------------
            # pose-major: partition = pose, free = (k, d) = 4*256
            for cat in range(C):
                for pt in range(NPL // 128):  # 2 pose tiles per cat
                    col4 = (cat * (NPL // 128) + pt) * K
                    po = pose_p.tile([128, D], mybir.dt.float32, tag="po")
                    nc.sync.dma_start(po[:], pose[cat, 128 * pt:128 * (pt + 1), :])
                    x = io.tile([128, K * D], mybir.dt.float32, tag="x")
                    nc.sync.dma_start(
                        x[:], nlp[cat, 512 * pt:512 * (pt + 1), :]
                        .rearrange("(p k) d -> p k d", k=K))
                    # replicate pose row 4x along the free axis
                    pb = bcast.tile([128, K * D], mybir.dt.float32, tag="pb")
                    full = po[:]
                    rep = bass.AP(tensor=full.tensor, offset=full.offset,
                                  ap=[list(full.ap[0]), [0, K], [1, D]])
                    nc.sync.dma_start(pb[:], rep)
                    z = scr.tile([128, K * D], mybir.dt.float32, tag="z")
                    nc.vector.tensor_tensor(z[:], x[:], pb[:],
                                            op=mybir.AluOpType.mult)
                    nc.vector.tensor_reduce(
                        rdot_t[:, col4:col4 + K],
                        z[:].rearrange("p (k d) -> p k d", k=K),
                        axis=mybir.AxisListType.X, op=mybir.AluOpType.add)
                    s1 = scr.tile([128, D], mybir.dt.float32, tag="s1")
                    for k in range(K):
                        nc.scalar.activation(s1[:], x[:, D * k:D * (k + 1)],
                                             mybir.ActivationFunctionType.Square,
                                             accum_out=ssq_t[:, col4 + k:col4 + k + 1])

            # ---- B) pm block matmul --------------------------------------
            if with_pm:
                KT = CD // 128  # 16 k tiles
                if PM_MODE == "bf16split":
                    lt_h = matres.tile([128, KT, PM_M], mybir.dt.bfloat16, tag="lt_h")
                    lt_l = matres.tile([128, KT, PM_M], mybir.dt.bfloat16, tag="lt_l")
                    nc.sync.dma_start(lt_h[:], pml_h.rearrange("(k p) m -> p k m", p=128))
                    nc.sync.dma_start(lt_l[:], pml_l.rearrange("(k p) m -> p k m", p=128))
                    for n in range(2):
                        accs = [ps.tile([128, 512], mybir.dt.float32, name=f"acc{n}{m}", tag=f"acc{n}{m}")
                                for m in range(PM_MI)]
                        for k in range(KT):
                            rh = rhs_p.tile([128, 512], mybir.dt.bfloat16, tag="rh")
                            rl = rhs_p.tile([128, 512], mybir.dt.bfloat16, tag="rl")
                            nc.sync.dma_start(rh[:], pmr_h[128 * k:128 * (k + 1), 512 * n:512 * (n + 1)])
                            nc.sync.dma_start(rl[:], pmr_l[128 * k:128 * (k + 1), 512 * n:512 * (n + 1)])
                            for m in range(PM_MI):
                                lh = lt_h[:, k, 128 * m:128 * (m + 1)]
                                ll = lt_l[:, k, 128 * m:128 * (m + 1)]
                                nc.tensor.matmul(accs[m][:], lh, rh[:], start=(k == 0), stop=False)
                                nc.tensor.matmul(accs[m][:], lh, rl[:], start=False, stop=False)
                                nc.tensor.matmul(accs[m][:], ll, rh[:], start=False, stop=(k == KT - 1))
                        for m in range(PM_MI):
                            o = ev.tile([128, 512], mybir.dt.float32, tag="ev")
                            nc.scalar.copy(o[:], accs[m][:])
                            nc.sync.dma_start(
                                pmblk[128 * m:128 * (m + 1), 512 * n:512 * (n + 1)], o[:])
                else:
                    dt_ = mybir.dt.float32r if PM_MODE == "f32r" else mybir.dt.float32
                    lt = matres.tile([128, KT, PM_M], dt_, tag="lt")
                    nc.sync.dma_start(lt[:], pm_l.rearrange("(k p) m -> p k m", p=128))
                    for n in range(2):
                        accs = [ps.tile([128, 512], mybir.dt.float32, name=f"acc{n}{m}", tag=f"acc{n}{m}")
                                for m in range(PM_MI)]
                        for k in range(KT):
                            rt = rhs_p.tile([128, 512], dt_, tag="rt")
                            nc.sync.dma_start(rt[:], pm_r[128 * k:128 * (k + 1), 512 * n:512 * (n + 1)])
                            for m in range(PM_MI):
                                nc.tensor.matmul(accs[m][:], lt[:, k, 128 * m:128 * (m + 1)], rt[:],
                                                 start=(k == 0), stop=(k == KT - 1))
                        for m in range(PM_MI):
                            o = ev.tile([128, 512], mybir.dt.float32, tag="ev")
                            nc.scalar.copy(o[:], accs[m][:])
                            nc.sync.dma_start(
                                pmblk[128 * m:128 * (m + 1), 512 * n:512 * (n + 1)], o[:])

            nc.sync.dma_start(ssq[:], ssq_t[:])
            nc.sync.dma_start(rdot[:], rdot_t[:])

    nc.finalize()
    return nc


def _make_runner(nc):
    """Reusable jitted SPMD runner (replicates bass2jax.run_bass_via_pjrt but
    caches the compiled executable across calls)."""
    import jax
    from jax.sharding import Mesh, PartitionSpec
    from jax.experimental.shard_map import shard_map

    install_neuronx_cc_hook()
    partition_name = nc.partition_id_tensor.name if nc.partition_id_tensor else None
    in_names, out_names, out_avals = [], [], []
    for alloc in nc.m.functions[0].allocations:
        if not isinstance(alloc, mybir.MemoryLocationSet):
            continue
        name = alloc.memorylocations[0].name
        if alloc.kind == "ExternalInput":
            if name != partition_name:
                in_names.append(name)
        elif alloc.kind == "ExternalOutput":
            out_names.append(name)
            out_avals.append(jax.core.ShapedArray(
                tuple(alloc.tensor_shape), mybir.dt.np(alloc.dtype)))
    n_params = len(in_names)
    all_in = in_names + out_names + ([partition_name] if partition_name else [])

    def _body(*args):
        operands = list(args)
        if partition_name is not None:
            operands.append(partition_id_tensor())
        outs = _bass_exec_p.bind(
            *operands, out_avals=tuple(out_avals), in_names=tuple(all_in),
            out_names=tuple(out_names), lowering_input_output_aliases=(),
            sim_require_finite=False, sim_require_nnan=False, nc=nc)
        return tuple(outs)

    devices = jax.devices()[:NCORES]
    mesh = Mesh(np.asarray(devices), ("core",))
    donate = tuple(range(n_params, n_params + len(out_names)))
    sharded = jax.jit(
        shard_map(_body, mesh=mesh,
                  in_specs=(PartitionSpec("core"),) * (n_params + len(out_names)),
                  out_specs=(PartitionSpec("core"),) * len(out_names),
                  check_rep=False),
        donate_argnums=donate, keep_unused=True)

    def run(in_maps):
        concat_in = [np.concatenate([np.asarray(m[name]) for m in in_maps], axis=0)
                     for name in in_names]
        zeros = [np.zeros((NCORES * a.shape[0], *a.shape[1:]), a.dtype)
                 for a in out_avals]
        out_arrs = sharded(*concat_in, *zeros)
        return [
            {name: np.asarray(out_arrs[i]).reshape(NCORES, *out_avals[i].shape)[c]
             for i, name in enumerate(out_names)}
            for c in range(NCORES)
        ]

    return run


def _get_runner(key):
    if key not in _runners:
        if key == "k1":
            _runners[key] = _make_runner(_build_dots_kernel(with_pm=True))
        else:
            _runners[key] = _make_runner(_build_dots_kernel(with_pm=False))
    return _runners[key]


def _to_bf16_pair(x):
    """Split f32 into bf16 hi + bf16 lo (hi = rne(x), lo = rne(x - hi))."""
    import ml_dtypes
    hi = x.astype(ml_dtypes.bfloat16)
    lo = (x - hi.astype(np.float32)).astype(ml_dtypes.bfloat16)
    return hi, lo


def _col_to_rows(a):
    """[8 cores][128, C*NT] device output -> [C, NN] (global nlp rows).

    column = (cat*2 + pt)*K + k ; partition p -> nlp row 512*pt + 4*p + k
    within the core shard (pose-major layout)."""
    out = np.empty((C, NN), np.float32)
    for c in range(NCORES):
        blk = a[c].reshape(128, C, NPL // 128, K)      # [p, cat, pt, k]
        out[:, c * NNL:(c + 1) * NNL] = (
            blk.transpose(1, 2, 0, 3).reshape(C, NNL))
    return out


def kernel(**inputs):
    nlp = np.ascontiguousarray(inputs["nlp_features"], np.float32)      # [C, NN, D]
    pose = np.ascontiguousarray(inputs["pose_features"], np.float32)    # [C, NP, D]
    nlab = np.asarray(inputs["nlp_label"]).astype(np.int64)
    cat = np.ascontiguousarray(inputs["categories"], np.float32)        # [NN, C]
    ri = np.asarray(inputs["rand_index"]).astype(np.int64)

    # ---- host: pose normalization + poseFT ------------------------------
    norm_p = np.sqrt(np.einsum("cpd,cpd->cp", pose, pose, dtype=np.float32,
                               optimize=True)).astype(np.float32)       # [C, NP]
    poseF = pose / norm_p[:, :, None]
    poseFT = np.ascontiguousarray(
        poseF.transpose(0, 2, 1).reshape(CD, NP))                       # [CD, NP]

    # ---- device kernel 1 -------------------------------------------------
    run1 = _get_runner("k1")
    in_maps = []
    for c in range(NCORES):
        i, j = c // PM_NJ, c % PM_NJ
        m = {
            "nlp": nlp[:, c * NNL:(c + 1) * NNL, :],
            "pose": poseF[:, c * NPL:(c + 1) * NPL, :],
        }
        lsl = poseFT[:, i * PM_M:(i + 1) * PM_M]
        rsl = poseFT[:, j * PM_N:(j + 1) * PM_N]
        if PM_MODE == "bf16split":
            m["pml_h"], m["pml_l"] = _to_bf16_pair(lsl)
            m["pmr_h"], m["pmr_l"] = _to_bf16_pair(rsl)
        else:
            m["pm_l"] = np.ascontiguousarray(lsl)
            m["pm_r"] = np.ascontiguousarray(rsl)
        in_maps.append(m)
    res1 = run1(in_maps)

    ssq_n = _col_to_rows([r["ssq"] for r in res1])                      # [C, NN]
    rdot = _col_to_rows([r["rdot"] for r in res1])                      # [C, NN]
    pm = np.empty((NP, NP), np.float32)
    for c in range(NCORES):
        i, j = c // PM_NJ, c % PM_NJ
        pm[i * PM_M:(i + 1) * PM_M, j * PM_N:(j + 1) * PM_N] = res1[c]["pmblk"]

    # ---- host: norms / scores / BCE -------------------------------------
    norm_n = np.sqrt(ssq_n)                                             # [C, NN]
    loss_norm = np.float32(np.float32(norm_p.mean()) + np.float32(norm_n.mean()))

    dots = (rdot / norm_n).astype(np.float32)                           # [C, NN]
    scores = np.einsum("cn,nc->n", dots, cat).astype(np.float32)
    p = (1.0 / (1.0 + np.exp(-scores))).astype(np.float32)
    lblf = nlab.astype(np.float32)
    loss_label = np.float32(
        np.mean(-(np.log(p) * lblf + np.log(1.0 - p) * (1.0 - lblf))))

    # ---- host: furthest selection ---------------------------------------
    ar = np.arange(NP)
    pm[ar, ar] = 1.0
    order = np.argsort(pm, axis=1, kind="stable")
    furthest = order[ar, ri]                                            # [NP]

    sg = scores.reshape(NP, K)
    lg = nlab.reshape(NP, K)
    maxp = np.maximum(np.max(np.where(lg == 0, sg, -np.inf), axis=1), -1.0)
    minp = np.minimum(np.min(np.where(lg == 1, sg, np.inf), axis=1), 1.0)

    nids = (furthest[:, None] * K + np.arange(K)).reshape(-1)           # [NN]

    # ---- device kernel 2: dots of gathered hard-positive rows ------------
    run2 = _get_runner("k2")
    gnlp = nlp[:, nids, :]                                              # [C, NN, D]
    in_maps2 = []
    for c in range(NCORES):
        in_maps2.append({
            "nlp": gnlp[:, c * NNL:(c + 1) * NNL, :],
            "pose": poseF[:, c * NPL:(c + 1) * NPL, :],
        })
    res2 = run2(in_maps2)
    crdot = _col_to_rows([r["rdot"] for r in res2])                     # [C, NN]

    cur_dots = (crdot / norm_n[:, nids]).astype(np.float32)             # [C, NN]
    cur = np.einsum("cn,nc->n", cur_dots, cat[nids]).astype(np.float32)
    cur = cur.reshape(NP, K)
    lcur = nlab[nids].reshape(NP, K)
    maxcur = np.max(np.where(lcur == 1, cur, -np.inf), axis=1)
    maxp = np.maximum(maxp, maxcur)
    found = ~((maxp == -1.0) | (minp == 1.0))
    lt = np.where(found, maxp - minp + 2.0, 0.0).astype(np.float32)
    not_find = int(np.sum(~found))
    if not_find == NN:
        loss_triple = np.float32(0.0)
    else:
        loss_triple = np.float32(lt.sum(dtype=np.float32) / np.float32(NN - not_find))

    return (np.float32(loss_label), np.float32(loss_norm), np.float32(loss_triple))


# revision 14
# speedup vs baseline: 1.5126x; 1.5126x over previous
"""Trainium2 Bass kernel for nn_ContrastLoss_Disentangle.

Contract: kernel(**inputs) takes the FULL (unsharded) inputs and returns the
same structure the reference returns: (loss_label, loss_norm, loss_triple)
as float32 scalars.

Pipeline (8 NeuronCores, data-parallel):
  host:    pose norms + normalization + [C*D, Np] transpose (poseFT)
  device1: per-core: nlp-row sumsq (ACT square+accum), raw nlp.pose dots
           (DVE mul + grouped reduce), and a [512, 1024] block of the
           pm gram matrix (PE, f32r fast path)
  host:    scores + BCE, pm assembly + stable argsort rank-select (furthest)
  device2: per-core dots of the gathered "hard positive" nlp rows
  host:    triplet loss assembly
"""

import os
import numpy as np

import concourse.bass as bass
import concourse.tile as tile
from concourse import bacc, mybir
from concourse.bass2jax import install_neuronx_cc_hook, partition_id_tensor, _bass_exec_p

C, NP, K, D = 8, 2048, 4, 256
NN = NP * K          # 8192
NCORES = 8
NPL = NP // NCORES   # 256 poses per core
NNL = NN // NCORES   # 1024 nlp rows per core
NT = NNL // 128      # 8 nlp tiles per category per core
CD = C * D           # 2048 contraction size

# pm block grid: 4 row-blocks x 2 col-blocks
PM_MI, PM_NJ = 4, 2
PM_M = NP // PM_MI   # 512 rows per core block
PM_N = NP // PM_NJ   # 1024 cols per core block

PM_MODE = os.environ.get("PM_MODE", "f32r")  # "f32r" | "bf16split" | "f32"

_runners = {}


def _build_dots_kernel(with_pm: bool, with_ssq: bool = True):
    """Per-core program. Inputs (per core):
      nlp   [C, NNL, D] f32   (raw nlp rows of this core; pose-major blocks)
      pose  [C, NPL, D] f32   (normalized pose rows matching this core's nlps)
      pm_l / pm_r             (poseFT column slices; only when with_pm)
    Outputs:
      ssq  [128, C*NT] f32    sumsq of nlp rows (col = (cat*2+pt)*4+k)
      rdot [128, C*NT] f32    dot(nlp_row, poseF[row//4]), same layout
      pmblk [PM_M, PM_N] f32  (only when with_pm)
    """
    nc = bacc.Bacc("TRN2", target_bir_lowering=False, debug=False,
                   num_devices=NCORES)
    nlp = nc.dram_tensor("nlp", [C, NNL, D], mybir.dt.float32,
                         kind="ExternalInput").ap()
    pose = nc.dram_tensor("pose", [C, NPL, D], mybir.dt.float32,
                          kind="ExternalInput").ap()
    if with_pm:
        if PM_MODE == "bf16split":
            pml_h = nc.dram_tensor("pml_h", [CD, PM_M], mybir.dt.bfloat16, kind="ExternalInput").ap()
            pml_l = nc.dram_tensor("pml_l", [CD, PM_M], mybir.dt.bfloat16, kind="ExternalInput").ap()
            pmr_h = nc.dram_tensor("pmr_h", [CD, PM_N], mybir.dt.bfloat16, kind="ExternalInput").ap()
            pmr_l = nc.dram_tensor("pmr_l", [CD, PM_N], mybir.dt.bfloat16, kind="ExternalInput").ap()
        else:
            pmdt = mybir.dt.float32r if PM_MODE == "f32r" else mybir.dt.float32
            pm_l = nc.dram_tensor("pm_l", [CD, PM_M], pmdt, kind="ExternalInput").ap()
            pm_r = nc.dram_tensor("pm_r", [CD, PM_N], pmdt, kind="ExternalInput").ap()
        pmblk = nc.dram_tensor("pmblk", [PM_M, PM_N], mybir.dt.float32,
                               kind="ExternalOutput").ap()
    if with_ssq:
        ssq = nc.dram_tensor("ssq", [128, C * NT], mybir.dt.float32,
                             kind="ExternalOutput").ap()
    rdot = nc.dram_tensor("rdot", [128, C * NT], mybir.dt.float32,
                          kind="ExternalOutput").ap()

    with tile.TileContext(nc) as tc:
        with tc.tile_pool(name="io", bufs=4) as io, \
             tc.tile_pool(name="pose_p", bufs=3) as pose_p, \
             tc.tile_pool(name="scr", bufs=4) as scr, \
             tc.tile_pool(name="accum", bufs=1) as accum, \
             tc.tile_pool(name="matres", bufs=1) as matres, \
             tc.tile_pool(name="rhs_p", bufs=4) as rhs_p, \
             tc.tile_pool(name="ev", bufs=3) as ev, \
             tc.tile_pool(name="ps", bufs=1, space="PSUM") as ps:

            if with_ssq:
                ssq_t = accum.tile([128, C * NT], mybir.dt.float32, tag="ssq")
                nc.gpsimd.memset(ssq_t[:], 0.0)
            rdot_t = accum.tile([128, C * NT], mybir.dt.float32, tag="rdot")
            nc.gpsimd.memset(rdot_t[:], 0.0)

            # ---------- sub-programs -------------------------------------
            def a_iter(i):
                cat, pt = i // 2, i % 2
                col4 = (cat * (NPL // 128) + pt) * K
                po = pose_p.tile([128, D], mybir.dt.float32, tag="po",
                                 name=f"po{i}")
                nc.sync.dma_start(po[:], pose[cat, 128 * pt:128 * (pt + 1), :])
                x = io.tile([128, K * D], mybir.dt.float32, tag="x",
                            name=f"x{i}")
                nc.sync.dma_start(
                    x[:], nlp[cat, 512 * pt:512 * (pt + 1), :]
                    .rearrange("(p k) d -> p k d", k=K))
                full = po[:]
                rep = bass.AP(tensor=full.tensor, offset=full.offset,
                              ap=[list(full.ap[0]), [0, K], [1, D]])
                z = scr.tile([128, K * D], mybir.dt.float32, tag="z",
                             name=f"z{i}")
                nc.vector.tensor_tensor(
                    z[:].rearrange("p (k d) -> p k d", k=K),
                    x[:].rearrange("p (k d) -> p k d", k=K),
                    rep, op=mybir.AluOpType.mult)
                if with_ssq:
                    # dots reduce on DVE, sumsq on ACT
                    nc.vector.tensor_reduce(
                        rdot_t[:, col4:col4 + K],
                        z[:].rearrange("p (k d) -> p k d", k=K),
                        axis=mybir.AxisListType.X, op=mybir.AluOpType.add)
                    s1 = scr.tile([128, D], mybir.dt.float32, tag="s1",
                                  name=f"s1_{i}")
                    for k in range(K):
                        nc.scalar.activation(
                            s1[:], x[:, D * k:D * (k + 1)],
                            mybir.ActivationFunctionType.Square,
                            accum_out=ssq_t[:, col4 + k:col4 + k + 1])
                elif i % 2 == 0:
                    # no sumsq needed -> alternate the dots reduce between
                    # the idle ACT (copy+accum) and the DVE (grouped reduce)
                    s1 = scr.tile([128, D], mybir.dt.float32, tag="s1",
                                  name=f"s1_{i}")
                    for k in range(K):
                        nc.scalar.activation(
                            s1[:], z[:, D * k:D * (k + 1)],
                            mybir.ActivationFunctionType.Copy,
                            accum_out=rdot_t[:, col4 + k:col4 + k + 1])
                else:
                    nc.vector.tensor_reduce(
                        rdot_t[:, col4:col4 + K],
                        z[:].rearrange("p (k d) -> p k d", k=K),
                        axis=mybir.AxisListType.X, op=mybir.AluOpType.add)

            accs = {}

            def pm_chunk(s):
                n, k = s // 16, s % 16
                KT = CD // 128
                if k == 0:
                    accs[n] = [ps.tile([128, 512], mybir.dt.float32,
                                       name=f"acc{n}{m}", tag=f"acc{n}{m}")
                               for m in range(PM_MI)]
                rt = rhs_p.tile([128, 512], pm_rdt, tag="rt", name=f"rt{s}")
                nc.sync.dma_start(rt[:], pm_r[128 * k:128 * (k + 1),
                                              512 * n:512 * (n + 1)])
                for m in range(PM_MI):
                    nc.tensor.matmul(accs[n][m][:],
                                     lt[:, k, 128 * m:128 * (m + 1)], rt[:],
                                     start=(k == 0), stop=(k == KT - 1))
                if k == KT - 1:
                    for m in range(PM_MI):
                        o = ev.tile([128, 512], mybir.dt.float32, tag="ev",
                                    name=f"ev{n}{m}")
                        nc.scalar.copy(o[:], accs[n][m][:])
                        nc.sync.dma_start(
                            pmblk[128 * m:128 * (m + 1),
                                  512 * n:512 * (n + 1)], o[:])

            # ---------- emission order: interleave pm with dots ----------
            if with_pm:
                assert PM_MODE in ("f32r", "f32"), "bf16split path removed"
                pm_rdt = mybir.dt.float32r if PM_MODE == "f32r" else mybir.dt.float32
                KT = CD // 128
                lt = matres.tile([128, KT, PM_M], pm_rdt, tag="lt")
                # split the big lhsT load into 4 chunks
                for kc in range(4):
                    nc.sync.dma_start(
                        lt[:, 4 * kc:4 * (kc + 1), :],
                        pm_l[512 * kc:512 * (kc + 1), :]
                        .rearrange("(k p) m -> p k m", p=128))
                for s in range(32):
                    pm_chunk(s)
                    if s % 2 == 0:
                        a_iter(s // 2)
            else:
                for i in range(16):
                    a_iter(i)

            if with_ssq:
                nc.sync.dma_start(ssq[:], ssq_t[:])
            nc.sync.dma_start(rdot[:], rdot_t[:])

    nc.finalize()
    return nc


def _make_runner(nc):
    """Reusable jitted SPMD runner (replicates bass2jax.run_bass_via_pjrt but
    caches the compiled executable across calls)."""
    import jax
    from jax.sharding import Mesh, PartitionSpec
    from jax.experimental.shard_map import shard_map

    install_neuronx_cc_hook()
    partition_name = nc.partition_id_tensor.name if nc.partition_id_tensor else None
    in_names, out_names, out_avals = [], [], []
    for alloc in nc.m.functions[0].allocations:
        if not isinstance(alloc, mybir.MemoryLocationSet):
            continue
        name = alloc.memorylocations[0].name
        if alloc.kind == "ExternalInput":
            if name != partition_name:
                in_names.append(name)
        elif alloc.kind == "ExternalOutput":
            out_names.append(name)
            out_avals.append(jax.core.ShapedArray(
                tuple(alloc.tensor_shape), mybir.dt.np(alloc.dtype)))
    n_params = len(in_names)
    all_in = in_names + out_names + ([partition_name] if partition_name else [])

    def _body(*args):
        operands = list(args)
        if partition_name is not None:
            operands.append(partition_id_tensor())
        outs = _bass_exec_p.bind(
            *operands, out_avals=tuple(out_avals), in_names=tuple(all_in),
            out_names=tuple(out_names), lowering_input_output_aliases=(),
            sim_require_finite=False, sim_require_nnan=False, nc=nc)
        return tuple(outs)

    devices = jax.devices()[:NCORES]
    mesh = Mesh(np.asarray(devices), ("core",))
    donate = tuple(range(n_params, n_params + len(out_names)))
    sharded = jax.jit(
        shard_map(_body, mesh=mesh,
                  in_specs=(PartitionSpec("core"),) * (n_params + len(out_names)),
                  out_specs=(PartitionSpec("core"),) * len(out_names),
                  check_rep=False),
        donate_argnums=donate, keep_unused=True)

    def run(in_maps):
        concat_in = [np.concatenate([np.asarray(m[name]) for m in in_maps], axis=0)
                     for name in in_names]
        zeros = [np.zeros((NCORES * a.shape[0], *a.shape[1:]), a.dtype)
                 for a in out_avals]
        out_arrs = sharded(*concat_in, *zeros)
        return [
            {name: np.asarray(out_arrs[i]).reshape(NCORES, *out_avals[i].shape)[c]
             for i, name in enumerate(out_names)}
            for c in range(NCORES)
        ]

    return run


def _get_runner(key):
    if key not in _runners:
        if key == "k1":
            _runners[key] = _make_runner(_build_dots_kernel(with_pm=True))
        else:
            _runners[key] = _make_runner(
                _build_dots_kernel(with_pm=False, with_ssq=False))
    return _runners[key]


def _col_to_rows(a):
    """[8 cores][128, C*NT] device output -> [C, NN] (global nlp rows).

    column = (cat*2 + pt)*K + k ; partition p -> nlp row 512*pt + 4*p + k
    within the core shard (pose-major layout)."""
    out = np.empty((C, NN), np.float32)
    for c in range(NCORES):
        blk = a[c].reshape(128, C, NPL // 128, K)      # [p, cat, pt, k]
        out[:, c * NNL:(c + 1) * NNL] = (
            blk.transpose(1, 2, 0, 3).reshape(C, NNL))
    return out


def kernel(**inputs):
    nlp = np.ascontiguousarray(inputs["nlp_features"], np.float32)      # [C, NN, D]
    pose = np.ascontiguousarray(inputs["pose_features"], np.float32)    # [C, NP, D]
    nlab = np.asarray(inputs["nlp_label"]).astype(np.int64)
    cat = np.ascontiguousarray(inputs["categories"], np.float32)        # [NN, C]
    ri = np.asarray(inputs["rand_index"]).astype(np.int64)

    # ---- host: pose normalization + poseFT ------------------------------
    norm_p = np.sqrt(np.einsum("cpd,cpd->cp", pose, pose, dtype=np.float32,
                               optimize=True)).astype(np.float32)       # [C, NP]
    poseF = pose / norm_p[:, :, None]
    poseFT = np.ascontiguousarray(
        poseF.transpose(0, 2, 1).reshape(CD, NP))                       # [CD, NP]

    # ---- device kernel 1 -------------------------------------------------
    run1 = _get_runner("k1")
    in_maps = []
    for c in range(NCORES):
        i, j = c // PM_NJ, c % PM_NJ
        m = {
            "nlp": nlp[:, c * NNL:(c + 1) * NNL, :],
            "pose": poseF[:, c * NPL:(c + 1) * NPL, :],
            "pm_l": np.ascontiguousarray(poseFT[:, i * PM_M:(i + 1) * PM_M]),
            "pm_r": np.ascontiguousarray(poseFT[:, j * PM_N:(j + 1) * PM_N]),
        }
        in_maps.append(m)
    res1 = run1(in_maps)

    ssq_n = _col_to_rows([r["ssq"] for r in res1])                      # [C, NN]
    rdot = _col_to_rows([r["rdot"] for r in res1])                      # [C, NN]
    pm = np.empty((NP, NP), np.float32)
    for c in range(NCORES):
        i, j = c // PM_NJ, c % PM_NJ
        pm[i * PM_M:(i + 1) * PM_M, j * PM_N:(j + 1) * PM_N] = res1[c]["pmblk"]

    # ---- host: norms / scores / BCE -------------------------------------
    norm_n = np.sqrt(ssq_n)                                             # [C, NN]
    loss_norm = np.float32(np.float32(norm_p.mean()) + np.float32(norm_n.mean()))

    dots = (rdot / norm_n).astype(np.float32)                           # [C, NN]
    scores = np.einsum("cn,nc->n", dots, cat).astype(np.float32)
    p = (1.0 / (1.0 + np.exp(-scores))).astype(np.float32)
    lblf = nlab.astype(np.float32)
    loss_label = np.float32(
        np.mean(-(np.log(p) * lblf + np.log(1.0 - p) * (1.0 - lblf))))

    # ---- host: furthest selection ---------------------------------------
    ar = np.arange(NP)
    pm[ar, ar] = 1.0
    order = np.argsort(pm, axis=1, kind="stable")
    furthest = order[ar, ri]                                            # [NP]

    sg = scores.reshape(NP, K)
    lg = nlab.reshape(NP, K)
    maxp = np.maximum(np.max(np.where(lg == 0, sg, -np.inf), axis=1), -1.0)
    minp = np.minimum(np.min(np.where(lg == 1, sg, np.inf), axis=1), 1.0)

    nids = (furthest[:, None] * K + np.arange(K)).reshape(-1)           # [NN]

    # ---- device kernel 2: dots of gathered hard-positive rows ------------
    run2 = _get_runner("k2")
    gnlp = nlp[:, nids, :]                                              # [C, NN, D]
    in_maps2 = []
    for c in range(NCORES):
        in_maps2.append({
            "nlp": gnlp[:, c * NNL:(c + 1) * NNL, :],
            "pose": poseF[:, c * NPL:(c + 1) * NPL, :],
        })
    res2 = run2(in_maps2)
    crdot = _col_to_rows([r["rdot"] for r in res2])                     # [C, NN]

    cur_dots = (crdot / norm_n[:, nids]).astype(np.float32)             # [C, NN]
    cur = np.einsum("cn,nc->n", cur_dots, cat[nids]).astype(np.float32)
    cur = cur.reshape(NP, K)
    lcur = nlab[nids].reshape(NP, K)
    maxcur = np.max(np.where(lcur == 1, cur, -np.inf), axis=1)
    maxp = np.maximum(maxp, maxcur)
    found = ~((maxp == -1.0) | (minp == 1.0))
    lt = np.where(found, maxp - minp + 2.0, 0.0).astype(np.float32)
    not_find = int(np.sum(~found))
    if not_find == NN:
        loss_triple = np.float32(0.0)
    else:
        loss_triple = np.float32(lt.sum(dtype=np.float32) / np.float32(NN - not_find))

    return (np.float32(loss_label), np.float32(loss_norm), np.float32(loss_triple))


# revision 19
# speedup vs baseline: 1.6145x; 1.0674x over previous
"""Trainium2 Bass kernel for nn_ContrastLoss_Disentangle.

Contract: kernel(**inputs) takes the FULL (unsharded) inputs and returns the
same structure the reference returns: (loss_label, loss_norm, loss_triple)
as float32 scalars.

Pipeline (8 NeuronCores, data-parallel):
  host:    pose norms + normalization + [C*D, Np] transpose (poseFT)
  device1: per-core: nlp-row sumsq (ACT square+accum), raw nlp.pose dots
           (DVE mul + grouped reduce), and a [512, 1024] block of the
           pm gram matrix (PE, f32r fast path)
  host:    scores + BCE, pm assembly + stable argsort rank-select (furthest)
  device2: per-core dots of the gathered "hard positive" nlp rows
  host:    triplet loss assembly
"""

import os
import numpy as np

import concourse.bass as bass
import concourse.tile as tile
from concourse import bacc, mybir
from concourse.bass2jax import install_neuronx_cc_hook, partition_id_tensor, _bass_exec_p

C, NP, K, D = 8, 2048, 4, 256
NN = NP * K          # 8192
NCORES = 8
NPL = NP // NCORES   # 256 poses per core
NNL = NN // NCORES   # 1024 nlp rows per core
NT = NNL // 128      # 8 nlp tiles per category per core
CD = C * D           # 2048 contraction size

# pm block grid: 4 row-blocks x 2 col-blocks
PM_MI, PM_NJ = 4, 2
PM_M = NP // PM_MI   # 512 rows per core block
PM_N = NP // PM_NJ   # 1024 cols per core block

PM_MODE = os.environ.get("PM_MODE", "f32r")  # "f32r" | "bf16split" | "f32"

_runners = {}


def _build_dots_kernel(with_pm: bool, with_ssq: bool = True):
    """Per-core program. Inputs (per core):
      nlp   [C, NNL, D] f32   (raw nlp rows of this core; pose-major blocks)
      pose  [C, NPL, D] f32   (normalized pose rows matching this core's nlps)
      pm_l / pm_r             (poseFT column slices; only when with_pm)
    Outputs:
      ssq  [128, C*NT] f32    sumsq of nlp rows (col = (cat*2+pt)*4+k)
      rdot [128, C*NT] f32    dot(nlp_row, poseF[row//4]), same layout
      pmblk [PM_M, PM_N] f32  (only when with_pm)
    """
    nc = bacc.Bacc("TRN2", target_bir_lowering=False, debug=False,
                   num_devices=NCORES)
    nlp = nc.dram_tensor("nlp", [C, NNL, D], mybir.dt.float32,
                         kind="ExternalInput").ap()
    pose = nc.dram_tensor("pose", [C, NPL, D], mybir.dt.float32,
                          kind="ExternalInput").ap()
    if with_pm:
        if PM_MODE == "bf16split":
            pml_h = nc.dram_tensor("pml_h", [CD, PM_M], mybir.dt.bfloat16, kind="ExternalInput").ap()
            pml_l = nc.dram_tensor("pml_l", [CD, PM_M], mybir.dt.bfloat16, kind="ExternalInput").ap()
            pmr_h = nc.dram_tensor("pmr_h", [CD, PM_N], mybir.dt.bfloat16, kind="ExternalInput").ap()
            pmr_l = nc.dram_tensor("pmr_l", [CD, PM_N], mybir.dt.bfloat16, kind="ExternalInput").ap()
        else:
            pmdt = mybir.dt.float32r if PM_MODE == "f32r" else mybir.dt.float32
            pm_l = nc.dram_tensor("pm_l", [CD, PM_M], pmdt, kind="ExternalInput").ap()
            pm_r = nc.dram_tensor("pm_r", [CD, PM_N], pmdt, kind="ExternalInput").ap()
        pmblk = nc.dram_tensor("pmblk", [PM_M, PM_N], mybir.dt.float32,
                               kind="ExternalOutput").ap()
    if with_ssq:
        ssq = nc.dram_tensor("ssq", [128, C * NT], mybir.dt.float32,
                             kind="ExternalOutput").ap()
    rdot = nc.dram_tensor("rdot", [128, C * NT], mybir.dt.float32,
                          kind="ExternalOutput").ap()

    with tile.TileContext(nc) as tc:
        with tc.tile_pool(name="io", bufs=int(os.environ.get("BUFS_IO", 6))) as io, \
             tc.tile_pool(name="pose_p", bufs=3) as pose_p, \
             tc.tile_pool(name="scr", bufs=4) as scr, \
             tc.tile_pool(name="accum", bufs=1) as accum, \
             tc.tile_pool(name="matres", bufs=1) as matres, \
             tc.tile_pool(name="rhs_p", bufs=int(os.environ.get("BUFS_RHS", 6))) as rhs_p, \
             tc.tile_pool(name="ev", bufs=3) as ev, \
             tc.tile_pool(name="ps", bufs=1, space="PSUM") as ps:

            if with_ssq:
                ssq_t = accum.tile([128, C * NT], mybir.dt.float32, tag="ssq")
                nc.gpsimd.memset(ssq_t[:], 0.0)
            rdot_t = accum.tile([128, C * NT], mybir.dt.float32, tag="rdot")
            nc.gpsimd.memset(rdot_t[:], 0.0)

            # ---------- sub-programs -------------------------------------
            def a_iter(i):
                cat, pt = i // 2, i % 2
                col4 = (cat * (NPL // 128) + pt) * K
                po = pose_p.tile([128, D], mybir.dt.float32, tag="po",
                                 name=f"po{i}")
                nc.sync.dma_start(po[:], pose[cat, 128 * pt:128 * (pt + 1), :])
                x = io.tile([128, K * D], mybir.dt.float32, tag="x",
                            name=f"x{i}")
                nc.sync.dma_start(
                    x[:], nlp[cat, 512 * pt:512 * (pt + 1), :]
                    .rearrange("(p k) d -> p k d", k=K))
                full = po[:]
                rep = bass.AP(tensor=full.tensor, offset=full.offset,
                              ap=[list(full.ap[0]), [0, K], [1, D]])
                z = scr.tile([128, K * D], mybir.dt.float32, tag="z",
                             name=f"z{i}")
                nc.vector.tensor_tensor(
                    z[:].rearrange("p (k d) -> p k d", k=K),
                    x[:].rearrange("p (k d) -> p k d", k=K),
                    rep, op=mybir.AluOpType.mult)
                if with_ssq:
                    # dots reduce on DVE, sumsq on ACT
                    nc.vector.tensor_reduce(
                        rdot_t[:, col4:col4 + K],
                        z[:].rearrange("p (k d) -> p k d", k=K),
                        axis=mybir.AxisListType.X, op=mybir.AluOpType.add)
                    s1 = scr.tile([128, D], mybir.dt.float32, tag="s1",
                                  name=f"s1_{i}")
                    for k in range(K):
                        nc.scalar.activation(
                            s1[:], x[:, D * k:D * (k + 1)],
                            mybir.ActivationFunctionType.Square,
                            accum_out=ssq_t[:, col4 + k:col4 + k + 1])
                elif i % 2 == 0:
                    # no sumsq needed -> alternate the dots reduce between
                    # the idle ACT (copy+accum) and the DVE (grouped reduce)
                    s1 = scr.tile([128, D], mybir.dt.float32, tag="s1",
                                  name=f"s1_{i}")
                    for k in range(K):
                        nc.scalar.activation(
                            s1[:], z[:, D * k:D * (k + 1)],
                            mybir.ActivationFunctionType.Copy,
                            accum_out=rdot_t[:, col4 + k:col4 + k + 1])
                else:
                    nc.vector.tensor_reduce(
                        rdot_t[:, col4:col4 + K],
                        z[:].rearrange("p (k d) -> p k d", k=K),
                        axis=mybir.AxisListType.X, op=mybir.AluOpType.add)

            accs = {}

            def pm_chunk(s):
                n, k = s // 16, s % 16
                KT = CD // 128
                if k == 0:
                    accs[n] = [ps.tile([128, 512], mybir.dt.float32,
                                       name=f"acc{n}{m}", tag=f"acc{n}{m}")
                               for m in range(PM_MI)]
                rt = rhs_p.tile([128, 512], pm_rdt, tag="rt", name=f"rt{s}")
                nc.sync.dma_start(rt[:], pm_r[128 * k:128 * (k + 1),
                                              512 * n:512 * (n + 1)])
                for m in range(PM_MI):
                    nc.tensor.matmul(accs[n][m][:],
                                     lt[:, k, 128 * m:128 * (m + 1)], rt[:],
                                     start=(k == 0), stop=(k == KT - 1))
                if k == KT - 1:
                    for m in range(PM_MI):
                        o = ev.tile([128, 512], mybir.dt.float32, tag="ev",
                                    name=f"ev{n}{m}")
                        nc.scalar.copy(o[:], accs[n][m][:])
                        nc.sync.dma_start(
                            pmblk[128 * m:128 * (m + 1),
                                  512 * n:512 * (n + 1)], o[:])

            # ---------- emission order: interleave pm with dots ----------
            if with_pm:
                assert PM_MODE in ("f32r", "f32"), "bf16split path removed"
                pm_rdt = mybir.dt.float32r if PM_MODE == "f32r" else mybir.dt.float32
                KT = CD // 128
                _order = os.environ.get("K1_ORDER", "A")
                # unblock DVE/ACT before the big lhsT load hits the queues
                a_iter(0)
                a_iter(1)
                lt = matres.tile([128, KT, PM_M], pm_rdt, tag="lt")
                # split the big lhsT load into 4 chunks
                for kc in range(4):
                    nc.sync.dma_start(
                        lt[:, 4 * kc:4 * (kc + 1), :],
                        pm_l[512 * kc:512 * (kc + 1), :]
                        .rearrange("(k p) m -> p k m", p=128))
                for s in range(32):
                    pm_chunk(s)
                    if _order == "A":
                        if s % 2 == 0 and 2 + s // 2 < 16:
                            a_iter(2 + s // 2)
                    else:
                        if 2 + s < 16:
                            a_iter(2 + s)
            else:
                for i in range(16):
                    a_iter(i)

            if with_ssq:
                nc.sync.dma_start(ssq[:], ssq_t[:])
            nc.sync.dma_start(rdot[:], rdot_t[:])

    nc.finalize()
    return nc


def _make_runner(nc):
    """Reusable jitted SPMD runner (replicates bass2jax.run_bass_via_pjrt but
    caches the compiled executable across calls)."""
    import jax
    from jax.sharding import Mesh, PartitionSpec
    from jax.experimental.shard_map import shard_map

    install_neuronx_cc_hook()
    partition_name = nc.partition_id_tensor.name if nc.partition_id_tensor else None
    in_names, out_names, out_avals = [], [], []
    for alloc in nc.m.functions[0].allocations:
        if not isinstance(alloc, mybir.MemoryLocationSet):
            continue
        name = alloc.memorylocations[0].name
        if alloc.kind == "ExternalInput":
            if name != partition_name:
                in_names.append(name)
        elif alloc.kind == "ExternalOutput":
            out_names.append(name)
            out_avals.append(jax.core.ShapedArray(
                tuple(alloc.tensor_shape), mybir.dt.np(alloc.dtype)))
    n_params = len(in_names)
    all_in = in_names + out_names + ([partition_name] if partition_name else [])

    def _body(*args):
        operands = list(args)
        if partition_name is not None:
            operands.append(partition_id_tensor())
        outs = _bass_exec_p.bind(
            *operands, out_avals=tuple(out_avals), in_names=tuple(all_in),
            out_names=tuple(out_names), lowering_input_output_aliases=(),
            sim_require_finite=False, sim_require_nnan=False, nc=nc)
        return tuple(outs)

    devices = jax.devices()[:NCORES]
    mesh = Mesh(np.asarray(devices), ("core",))
    donate = tuple(range(n_params, n_params + len(out_names)))
    sharded = jax.jit(
        shard_map(_body, mesh=mesh,
                  in_specs=(PartitionSpec("core"),) * (n_params + len(out_names)),
                  out_specs=(PartitionSpec("core"),) * len(out_names),
                  check_rep=False),
        donate_argnums=donate, keep_unused=True)

    def run(in_maps):
        concat_in = [np.concatenate([np.asarray(m[name]) for m in in_maps], axis=0)
                     for name in in_names]
        zeros = [np.zeros((NCORES * a.shape[0], *a.shape[1:]), a.dtype)
                 for a in out_avals]
        out_arrs = sharded(*concat_in, *zeros)
        return [
            {name: np.asarray(out_arrs[i]).reshape(NCORES, *out_avals[i].shape)[c]
             for i, name in enumerate(out_names)}
            for c in range(NCORES)
        ]

    return run


def _get_runner(key):
    if key not in _runners:
        if key == "k1":
            _runners[key] = _make_runner(_build_dots_kernel(with_pm=True))
        else:
            _runners[key] = _make_runner(
                _build_dots_kernel(with_pm=False, with_ssq=False))
    return _runners[key]


def _col_to_rows(a):
    """[8 cores][128, C*NT] device output -> [C, NN] (global nlp rows).

    column = (cat*2 + pt)*K + k ; partition p -> nlp row 512*pt + 4*p + k
    within the core shard (pose-major layout)."""
    out = np.empty((C, NN), np.float32)
    for c in range(NCORES):
        blk = a[c].reshape(128, C, NPL // 128, K)      # [p, cat, pt, k]
        out[:, c * NNL:(c + 1) * NNL] = (
            blk.transpose(1, 2, 0, 3).reshape(C, NNL))
    return out


def kernel(**inputs):
    nlp = np.ascontiguousarray(inputs["nlp_features"], np.float32)      # [C, NN, D]
    pose = np.ascontiguousarray(inputs["pose_features"], np.float32)    # [C, NP, D]
    nlab = np.asarray(inputs["nlp_label"]).astype(np.int64)
    cat = np.ascontiguousarray(inputs["categories"], np.float32)        # [NN, C]
    ri = np.asarray(inputs["rand_index"]).astype(np.int64)

    # ---- host: pose normalization + poseFT ------------------------------
    norm_p = np.sqrt(np.einsum("cpd,cpd->cp", pose, pose, dtype=np.float32,
                               optimize=True)).astype(np.float32)       # [C, NP]
    poseF = pose / norm_p[:, :, None]
    poseFT = np.ascontiguousarray(
        poseF.transpose(0, 2, 1).reshape(CD, NP))                       # [CD, NP]

    # ---- device kernel 1 -------------------------------------------------
    run1 = _get_runner("k1")
    in_maps = []
    for c in range(NCORES):
        i, j = c // PM_NJ, c % PM_NJ
        m = {
            "nlp": nlp[:, c * NNL:(c + 1) * NNL, :],
            "pose": poseF[:, c * NPL:(c + 1) * NPL, :],
            "pm_l": np.ascontiguousarray(poseFT[:, i * PM_M:(i + 1) * PM_M]),
            "pm_r": np.ascontiguousarray(poseFT[:, j * PM_N:(j + 1) * PM_N]),
        }
        in_maps.append(m)
    res1 = run1(in_maps)

    ssq_n = _col_to_rows([r["ssq"] for r in res1])                      # [C, NN]
    rdot = _col_to_rows([r["rdot"] for r in res1])                      # [C, NN]
    pm = np.empty((NP, NP), np.float32)
    for c in range(NCORES):
        i, j = c // PM_NJ, c % PM_NJ
        pm[i * PM_M:(i + 1) * PM_M, j * PM_N:(j + 1) * PM_N] = res1[c]["pmblk"]

    # ---- host: norms / scores / BCE -------------------------------------
    norm_n = np.sqrt(ssq_n)                                             # [C, NN]
    loss_norm = np.float32(np.float32(norm_p.mean()) + np.float32(norm_n.mean()))

    dots = (rdot / norm_n).astype(np.float32)                           # [C, NN]
    scores = np.einsum("cn,nc->n", dots, cat).astype(np.float32)
    p = (1.0 / (1.0 + np.exp(-scores))).astype(np.float32)
    lblf = nlab.astype(np.float32)
    loss_label = np.float32(
        np.mean(-(np.log(p) * lblf + np.log(1.0 - p) * (1.0 - lblf))))

    # ---- host: furthest selection ---------------------------------------
    ar = np.arange(NP)
    pm[ar, ar] = 1.0
    order = np.argsort(pm, axis=1, kind="stable")
    furthest = order[ar, ri]                                            # [NP]

    sg = scores.reshape(NP, K)
    lg = nlab.reshape(NP, K)
    maxp = np.maximum(np.max(np.where(lg == 0, sg, -np.inf), axis=1), -1.0)
    minp = np.minimum(np.min(np.where(lg == 1, sg, np.inf), axis=1), 1.0)

    nids = (furthest[:, None] * K + np.arange(K)).reshape(-1)           # [NN]

    # ---- device kernel 2: dots of gathered hard-positive rows ------------
    run2 = _get_runner("k2")
    gnlp = nlp[:, nids, :]                                              # [C, NN, D]
    in_maps2 = []
    for c in range(NCORES):
        in_maps2.append({
            "nlp": gnlp[:, c * NNL:(c + 1) * NNL, :],
            "pose": poseF[:, c * NPL:(c + 1) * NPL, :],
        })
    res2 = run2(in_maps2)
    crdot = _col_to_rows([r["rdot"] for r in res2])                     # [C, NN]

    cur_dots = (crdot / norm_n[:, nids]).astype(np.float32)             # [C, NN]
    cur = np.einsum("cn,nc->n", cur_dots, cat[nids]).astype(np.float32)
    cur = cur.reshape(NP, K)
    lcur = nlab[nids].reshape(NP, K)
    maxcur = np.max(np.where(lcur == 1, cur, -np.inf), axis=1)
    maxp = np.maximum(maxp, maxcur)
    found = ~((maxp == -1.0) | (minp == 1.0))
    lt = np.where(found, maxp - minp + 2.0, 0.0).astype(np.float32)
    not_find = int(np.sum(~found))
    if not_find == NN:
        loss_triple = np.float32(0.0)
    else:
        loss_triple = np.float32(lt.sum(dtype=np.float32) / np.float32(NN - not_find))

    return (np.float32(loss_label), np.float32(loss_norm), np.float32(loss_triple))
